# revision 12
# baseline (speedup 1.0000x reference)
"""Chamfer-distance (CDLoss) kernel for Trainium2, 8 NeuronCores.

Problem: B=16 point clouds x N=4096 points x D=3 (xyz), squared-L2 chamfer
distance with mean point/batch reduction (pytorch3d defaults); inputs are
flat [B*N, 3] with a sorted `batch` assignment vector.

Strategy (per the spec sharding hint): data-parallel over clouds, 2 clouds
per core.  Per cloud the 4096x4096 squared-distance matrix is produced on
the TensorEngine in PSUM tiles via a single matmul per tile with an
augmented contraction (zero-padded to K=128):

    d^2(p,q) = |p|^2 + |q|^2 - 2 p.q

Every host-side quantity is split into an fp16 hi+lo pair (v = hi + lo with
|lo| <= |v|*2^-11), so each fp16xfp16 product pairing contributes exactly
and only the O(2^-22) xl*yl cross terms are dropped: per coordinate k the
cross term -2*x_k*y_k uses three rows (-2xh*yh, -2xh*yl, -2xl*yh), and the
norms use hi/lo rows against ones -> K=13 used rows.  This matters: the
clouds' nearest-neighbour d^2 are ~1e-3 while plain-fp16 coordinate
rounding injects ~1e-3 absolute error into d^2 (it biased the min 40% low).
The matmul cost only depends on the free dim, so the extra rows are free;
the PE accumulates in fp32.  The matrix never touches HBM (flash-style):
 - ScalarE copies each PSUM tile to an fp16 SBUF stage,
 - VectorE min-reduces the stage along the free axis (per-row min -> cham_x)
 - VectorE elementwise-min folds the stage into a column accumulator
   (min across x-blocks -> cham_y direction), fp16 at 2x DVE rate.
Per-cloud row-mins [128,32] fp32 and column accumulators [128,4096] fp16
are DMA'd out; the final partition-axis min and the means happen on host.

This container's walrus only accepts ONE sync-wait per instruction, while
Tile emits multi-wait sync_info; _split_multi_waits() hoists extra waits
onto standalone NoOps on the same engine (semantically identical: engines
dispatch in order, so blocking earlier is strictly conservative).
"""

import numpy as np

B = 16
N = 4096
D = 3
NCORES = 8
CPC = B // NCORES  # clouds per core = 2
P = 128
NXB = N // P  # 32 x-blocks per cloud
KAUG = 16    # augmented rows actually used (13) padded to 16 for the host array
FDH = 2048   # PSUM tile free dim (4 banks)
MM_FD = 512  # single-matmul free dim (1 PSUM bank)
# (xb, h) PSUM halves drained by DVE instead of Act: empirically any DVE
# drain lengthens the critical path (sim), so keep empty.
SOLO_HALVES = set()

_cached = {}


def _split_multi_waits(nc):
    """Walrus in this container supports a single sync-wait per instruction;
    split any multi-wait sync_info into preceding single-wait NoOps."""
    import concourse.mybir as mybir

    for fn in nc.m.functions:
        for blk in fn.blocks:
            insts = blk.instructions
            out = []
            for inst in insts:
                si = inst.sync_info
                if si is not None and si.on_wait and len(si.on_wait) > 1:
                    waits = list(si.on_wait)
                    for j, w in enumerate(waits[:-1]):
                        nop = mybir.InstNoOp(
                            name=f"{inst.name}-wsp{j}",
                            engine=inst.engine,
                            ins=[],
                            outs=[],
                        )
                        nop.sync_info = mybir.SyncInfo(on_wait=[w], on_update=[])
                        out.append(nop)
                    si.on_wait = waits[-1:]
                out.append(inst)
            insts[:] = out


def _build_nc(reps=1):
    """reps>1 wraps the compute in a hardware For_i loop (identical results —
    min is idempotent); used only to amplify device time for wall-clock
    calibration of HW exec time."""
    import concourse.bass as bass
    import concourse.mybir as mybir
    import concourse.tile as tile
    from contextlib import nullcontext

    nc = bass.Bass()
    f16 = mybir.dt.float16
    f32 = mybir.dt.float32

    xt = nc.dram_tensor("xt", [CPC, KAUG, N], f16, kind="ExternalInput")
    yt = nc.dram_tensor("yt", [CPC, KAUG, N], f16, kind="ExternalInput")
    rowm = nc.dram_tensor("rowm", [CPC, P, 2 * NXB], f32, kind="ExternalOutput")
    colm = nc.dram_tensor("colm", [CPC, P, N], f16, kind="ExternalOutput")

    with tile.TileContext(nc) as tc:
        with (
            tc.tile_pool(name="singles", bufs=1) as singles,
            tc.tile_pool(name="stagep", bufs=2) as stagep,
            tc.tile_pool(name="scratchp", bufs=2) as scratchp,
            tc.tile_pool(name="accs", bufs=2) as accs,
            tc.tile_pool(name="psump", bufs=2, space="PSUM") as psump,
        ):
            # augmented inputs, one [KAUG, N] tile per cloud: the matmuls
            # contract over K=KAUG partitions directly (no zero padding --
            # PE time depends only on the free dim).
            xs, ys = [], []
            for c in range(CPC):
                xa = singles.tile([KAUG, N], f16, name=f"xa{c}")
                ya = singles.tile([KAUG, N], f16, name=f"ya{c}")
                nc.sync.dma_start(out=xa, in_=xt[c])
                nc.sync.dma_start(out=ya, in_=yt[c])
                xs.append(xa)
                ys.append(ya)

            rep_ctx = tc.For_i(0, reps, 1) if reps > 1 else nullcontext()
            with rep_ctx:
              colaccs, rowminss = [], []
              for c in range(CPC):
                pair = []
                for parity in range(2):
                    colacc = accs.tile(
                        [P, N], f16, name=f"colacc{c}_{parity}", tag="colacc"
                    )
                    nc.gpsimd.memset(colacc, 60000.0)
                    pair.append(colacc)
                rowmins = accs.tile(
                    [P, 2 * NXB], f32, name=f"rowmins{c}", tag="rowmins"
                )
                nc.gpsimd.memset(rowmins, 60000.0)
                colaccs.append(pair)
                rowminss.append(rowmins)
              # Interleave the two clouds' tiles: independent colacc/rowmins
              # chains keep every engine fed through the other cloud's
              # dependency stalls.
              for xb2 in range(CPC * NXB):
                c, xb = xb2 % CPC, xb2 // CPC
                xa, ya = xs[c], ys[c]
                colacc, rowmins = colaccs[c][xb % 2], rowminss[c]
                if True:
                    stage = stagep.tile([P, N], f16, name="stage", tag="stage")
                    any_solo = False
                    for h in range(N // FDH):
                        ps = psump.tile([P, FDH], f32, name="ps", tag="ps")
                        for k in range(FDH // MM_FD):
                            off = h * FDH + k * MM_FD
                            nc.tensor.matmul(
                                ps[:, k * MM_FD : (k + 1) * MM_FD],
                                lhsT=xa[:, xb * P : (xb + 1) * P],
                                rhs=ya[:, off : off + MM_FD],
                                start=True,
                                stop=True,
                            )
                        if (xb, h) in SOLO_HALVES:
                            # DVE drains PSUM itself (1x, fp32 in) while also
                            # producing this half's row-min; frees Act.
                            any_solo = True
                            nc.vector.tensor_scalar(
                                out=stage[:, h * FDH : (h + 1) * FDH],
                                in0=ps,
                                scalar1=1.0,
                                scalar2=None,
                                op0=mybir.AluOpType.mult,
                                op1=mybir.AluOpType.min,
                                accum_out=rowmins[:, 2 * xb + 1 : 2 * xb + 2],
                            )
                        else:
                            nc.scalar.copy(stage[:, h * FDH : (h + 1) * FDH], ps)
                    # column accumulator first (needs the full-width stage)
                    nc.vector.tensor_tensor(
                        out=colacc,
                        in0=stage,
                        in1=colacc,
                        op=mybir.AluOpType.min,
                    )
                    # row direction: one fused tensor_scalar whose accum
                    # min-reduces the stage along the free axis; the main
                    # output (identity copy) lands in a scratch buffer.
                    # Runs in the DVE 4x_2p perf mode (all-SBUF fp16).
                    # Solo halves already produced their own row-min; reduce
                    # only the Act-copied span then.
                    if any_solo:
                        lo, hi = FDH, N  # solo half is h=0 by construction
                    else:
                        lo, hi = 0, N
                    scratch = scratchp.tile([P, N], f16, name="scratch", tag="scratch")
                    nc.vector.tensor_scalar(
                        out=scratch[:, lo:hi],
                        in0=stage[:, lo:hi],
                        scalar1=1.0,
                        scalar2=None,
                        op0=mybir.AluOpType.mult,
                        op1=mybir.AluOpType.min,
                        accum_out=rowmins[:, 2 * xb : 2 * xb + 1],
                    )

              for c in range(CPC):
                nc.vector.tensor_tensor(
                    out=colaccs[c][0],
                    in0=colaccs[c][1],
                    in1=colaccs[c][0],
                    op=mybir.AluOpType.min,
                )
                nc.sync.dma_start(out=rowm[c], in_=rowminss[c])
                nc.sync.dma_start(out=colm[c], in_=colaccs[c][0])

    _split_multi_waits(nc)
    return nc


def _get_nc():
    if "nc" not in _cached:
        _cached["nc"] = _build_nc()
    return _cached["nc"]


def _to_dense(x, batch):
    """Mirror of torch_geometric to_dense_batch with static N, zero padding."""
    T = x.shape[0]
    b = batch.astype(np.int64)
    counts = np.bincount(b, minlength=B)
    starts = np.concatenate([[0], np.cumsum(counts)[:-1]]).astype(np.int64)
    pos = np.arange(T, dtype=np.int64) - starts[b]
    dense = np.zeros((B, N, x.shape[1]), dtype=np.float32)
    dense[b, pos] = x
    return dense


def _hi_lo(v):
    """fp64/fp32 array -> (hi, lo) fp16 pair with hi+lo ~= v to ~2^-22."""
    hi = v.astype(np.float16)
    lo = (v - hi.astype(np.float64)).astype(np.float16)
    return hi, lo


def _augment(dense, is_x):
    """dense [B,N,3] f32 -> [B,KAUG,N] f16 augmented rows.

    Row layout (both sides):  rows 3k,3k+1,3k+2 for coordinate k's cross
    term, rows 9..12 for the norm terms:
        x side: [-2xh, -2xh, -2xl]*3, nxh, nxl, 1, 1
        y side: [ yh,   yl,   yh]*3,   1,   1, nyh, nyl
    """
    d64 = dense.astype(np.float64)
    n2 = (d64 * d64).sum(axis=2)  # [B,N] fp64
    nh, nl = _hi_lo(n2)
    out = np.zeros((B, KAUG, N), dtype=np.float16)
    coords = np.swapaxes(d64, 1, 2)  # [B,3,N]
    ch, cl = _hi_lo(coords)
    if is_x:
        for k in range(3):
            m2h = (-2.0 * ch[:, k]).astype(np.float16)  # exact (scale by 2)
            m2l = (-2.0 * cl[:, k]).astype(np.float16)
            out[:, 3 * k + 0] = m2h
            out[:, 3 * k + 1] = m2h
            out[:, 3 * k + 2] = m2l
        out[:, 9] = nh
        out[:, 10] = nl
        out[:, 11] = 1.0
        out[:, 12] = 1.0
    else:
        for k in range(3):
            out[:, 3 * k + 0] = ch[:, k]
            out[:, 3 * k + 1] = cl[:, k]
            out[:, 3 * k + 2] = ch[:, k]
        out[:, 9] = 1.0
        out[:, 10] = 1.0
        out[:, 11] = nh
        out[:, 12] = nl
    return out


def kernel(pred, target, batch):
    from concourse.bass_utils import run_bass_kernel_spmd

    pred = np.asarray(pred)
    target = np.asarray(target)
    batch = np.asarray(batch)

    dense_x = _to_dense(pred.astype(np.float32), batch)
    dense_y = _to_dense(target.astype(np.float32), batch)

    xa = _augment(dense_x, is_x=True)   # [B,KAUG,N] f16
    ya = _augment(dense_y, is_x=False)  # [B,KAUG,N] f16

    in_maps = [
        {
            "xt": np.ascontiguousarray(xa[i * CPC : (i + 1) * CPC]),
            "yt": np.ascontiguousarray(ya[i * CPC : (i + 1) * CPC]),
        }
        for i in range(NCORES)
    ]

    nc = _get_nc()
    res = run_bass_kernel_spmd(nc, in_maps, core_ids=list(range(NCORES)))

    total = 0.0
    for i in range(NCORES):
        rowmv = res.results[i]["rowm"]  # [CPC,128,64] f32, per-half row mins
        colmv = res.results[i]["colm"]  # [CPC,128,4096] f16, col accumulators
        for c in range(CPC):
            rm = np.minimum(rowmv[c][:, 0::2], rowmv[c][:, 1::2])
            rowsum = rm.astype(np.float64).sum()
            colsum = colmv[c].astype(np.float32).min(axis=0).astype(np.float64).sum()
            total += rowsum + colsum

    return np.float32(total / (N * B))

